# revision 71
# baseline (speedup 1.0000x reference)
"""Trainium2 Bass kernel for BinaryTokenClassificationModel (segment_reduce).

Reference semantics (B=16, L=2048, H=1024, W=1024):
    src = segment_mean(hidden, source_word_ids)   # [B,W,H]
    tgt = segment_mean(hidden, target_word_ids)   # [B,W,H]
    logits[b,s,t,0] = src[b,s]@w_s + tgt[b,t]@w_t + bias

The pooled [B,W,H] tensors are never materialized: because the classifier
is linear, src_proj[b,s] = segment_mean_s(hidden[b,l] @ w_s), so only the
per-token scalar dots are needed.

The dots are computed on the TensorEngine: the host uploads hidden
pre-transposed/swizzled to token-group-major [p, g, c, tl] bf16 blocks,
and the PE contracts h-chunks of 128 against the stationary [128, 2]
classifier-weight tile, producing dots [2, L] in PSUM.  Tiny PE matmuls
against a 2x2 identity transpose the dots back to token-on-partition
layout, where one-hot segment matmuls (word = 128*q + r factorization)
reduce them per word.  The [W, W] output is emitted as an outer
broadcast-sum in bf16 (halves the store traffic; tolerance is 2e-2).

Sharding: data-parallel over batch - 2 examples per NeuronCore on 8 cores.
"""

from contextlib import ExitStack

import ml_dtypes
import numpy as np

import concourse.mybir as mybir
import concourse.tile as tile
from concourse import bacc
from concourse.bass_utils import run_bass_kernel_spmd
from concourse.masks import make_identity

P = 128          # partitions
B = 16           # full batch
NCORES = 8
BLOC = B // NCORES   # batches per core = 2
L = 2048         # tokens
H = 1024         # hidden
W = 1024         # words
Q = W // P       # 8 word chunks
NI = L // P      # 16 token groups per batch (token l = i*P + p)
HC = H // P      # 8 hidden chunks (h = c*P + p)
NG = 4           # dot matmul groups per batch (512 tokens, 1 MiB DMA each)
GL = L // NG

F32 = mybir.dt.float32
BF16 = mybir.dt.bfloat16
I32 = mybir.dt.int32

_CACHE = {}


def _build_module():
    nc = bacc.Bacc(None, target_bir_lowering=False, debug=False)
    names = {}
    with tile.TileContext(nc) as tc, ExitStack() as ctx:
        dram = ctx.enter_context(tc.tile_pool(name="dram", bufs=1, space="DRAM"))
        sb_c = ctx.enter_context(tc.tile_pool(name="const", bufs=1))
        sb_h = ctx.enter_context(tc.tile_pool(name="hid", bufs=16))
        sb_s = ctx.enter_context(tc.tile_pool(name="small", bufs=2))
        sb_o = ctx.enter_context(tc.tile_pool(name="outp", bufs=6))
        ps_dot = ctx.enter_context(tc.tile_pool(name="psdot", bufs=1, space="PSUM"))
        ps_seg = ctx.enter_context(tc.tile_pool(name="psseg", bufs=1, space="PSUM"))
        ps_sm = ctx.enter_context(tc.tile_pool(name="pssm", bufs=1, space="PSUM"))

        # hidT layout, token-group-major: [p, g, c, tl] =
        # hidden[g*GL + tl, c*128 + p], bf16, host-swizzled.  One 1 MiB DMA
        # per (batch, group) so dot compute trails the stream per group.
        hid_d = [dram.tile([P, NG, HC, GL], BF16, kind="ExternalInput",
                           name=f"hidT{b}")
                 for b in range(BLOC)]
        # all four id vectors, host-swizzled to [p, k, i] = ids[k, i*128+p]
        # (k = 2*b + side) so the DMA reads contiguous per-partition lines
        ids_d = dram.tile([P, 2 * BLOC, NI], I32, kind="ExternalInput")
        # packed classifier: cols 2c+s = w[side s, c*128+p], col 16 = bias
        wb_d = dram.tile([P, 2 * HC + 1], F32, kind="ExternalInput")
        out_d = [dram.tile([W, W], BF16, kind="ExternalOutput", name=f"logits{b}")
                 for b in range(BLOC)]

        names["hid"] = [t.name for t in hid_d]
        names["ids"] = ids_d.name
        names["wb"] = wb_d.name
        names["out"] = [t.name for t in out_d]

        # ---- small inputs FIRST on the sync ring: measured to land ~4us
        # this way vs ~9us when riding behind the flood ----
        wb = sb_c.tile([P, 2 * HC + 1], F32, tag="wb")
        nc.sync.dma_start(out=wb[:], in_=wb_d[:])
        ids_all = sb_c.tile([P, 2 * BLOC, NI], I32, tag="ids")
        nc.sync.dma_start(out=ids_all[:], in_=ids_d[:])
        w2f = wb[:, 0:2 * HC].rearrange("p (c s) -> p c s", s=2)
        w2 = sb_c.tile([P, HC, 2], BF16, tag="w2")
        nc.vector.tensor_copy(out=w2[:], in_=w2f)
        b_bc = wb[:, 2 * HC:2 * HC + 1]

        # identity FIRST: iota would otherwise stall make_identity ->
        # ident_bf -> the whole DVE chain on the GpSimd FIFO
        ident = sb_c.tile([P, P], F32, tag="id")
        make_identity(nc, ident[:])
        ident_bf = sb_c.tile([P, P], BF16, tag="idbf")
        nc.vector.tensor_copy(out=ident_bf[:], in_=ident[:])
        ones_bf = sb_c.tile([P, P], BF16, tag="ones")
        nc.vector.memset(ones_bf[:], 1.0)
        # iota_r16[p, 0, r] = r ; iota_q16[p, 0, q] = q  (broadcast along i
        # in the one-hot compares; [P,1,*] keeps iota ~10x cheaper)
        iota_r16 = sb_c.tile([P, 1, P], BF16, tag="ior")
        nc.gpsimd.iota(iota_r16[:], pattern=[[0, 1], [1, P]], base=0,
                       channel_multiplier=0, allow_small_or_imprecise_dtypes=True)
        iota_q16 = sb_c.tile([P, 1, Q], BF16, tag="ioq")
        nc.gpsimd.iota(iota_q16[:], pattern=[[0, 1], [1, Q]], base=0,
                       channel_multiplier=0, allow_small_or_imprecise_dtypes=True)

        # ---- ids -> (q, r) one-hots for BOTH batches, batched DVE ops ----
        q_i = sb_c.tile([P, 2 * BLOC, NI], I32, tag="qi")
        r_i = sb_c.tile([P, 2 * BLOC, NI], I32, tag="ri")
        nc.vector.tensor_scalar(out=q_i[:], in0=ids_all[:], scalar1=7,
                                scalar2=None,
                                op0=mybir.AluOpType.logical_shift_right)
        nc.vector.tensor_scalar(out=r_i[:], in0=ids_all[:], scalar1=127,
                                scalar2=None,
                                op0=mybir.AluOpType.bitwise_and)
        qf_all = sb_c.tile([P, 2 * BLOC, NI], BF16, tag="qfall")
        rf_all = sb_c.tile([P, 2 * BLOC, NI], BF16, tag="rfall")
        nc.vector.tensor_copy(out=qf_all[:], in_=q_i[:])
        nc.vector.tensor_copy(out=rf_all[:], in_=r_i[:])

        or_all_b = [None] * BLOC
        mdoq_b = [None] * BLOC
        for b in range(BLOC):
            or_all_b[b] = {}
            mdoq_b[b] = {}
            for sidx, side in enumerate(("s", "t")):
                k = 2 * b + sidx
                or_all_b[b][side] = sb_s.tile([P, NI, P], BF16,
                                              tag=f"orall{side}{b}",
                                              name=f"orall{side}{b}")
                nc.vector.tensor_tensor(
                    out=or_all_b[b][side][:],
                    in0=iota_r16[:].to_broadcast([P, NI, P]),
                    in1=rf_all[:, k, :].to_broadcast([P, NI, P]),
                    op=mybir.AluOpType.is_equal)
                mdoq_b[b][side] = sb_s.tile([P, NI, 2 * Q], BF16,
                                            tag=f"mdoq{side}{b}",
                                            name=f"mdoq{side}{b}")
                nc.vector.tensor_tensor(
                    out=mdoq_b[b][side][:, :, Q:2 * Q],
                    in0=iota_q16[:].to_broadcast([P, NI, Q]),
                    in1=qf_all[:, k, :].to_broadcast([P, NI, Q]),
                    op=mybir.AluOpType.is_equal)

        # ---- hidden streams for both batches (sync queue, deep prefetch) ----
        htg = [[None] * NG for _ in range(BLOC)]
        for b in range(BLOC):
            for g in range(NG):
                ht = sb_h.tile([P, HC, GL], BF16, tag="htg")
                nc.sync.dma_start(out=ht[:], in_=hid_d[b][:, g, :, :])
                htg[b][g] = ht

        # all four (batch, side) seg transposes land in ONE PSUM bank at
        # distinct column regions, so nothing serializes on buffer reuse
        seg4 = ps_sm.tile([P, 2 * BLOC * 2 * Q], F32, space="PSUM",
                          tag="seg4", name="seg4")

        GI = NI // NG  # token i-tiles per group = 4
        for b in range(BLOC):
            or_all = or_all_b[b]
            mdoq = mdoq_b[b]
            # both sides' segment sums share one PSUM bank:
            # cols 0:P = side s, P:2P = side t
            segT_both = ps_seg.tile([2 * Q, 2 * P], F32, space="PSUM",
                                    tag="segT", name="segT")
            segT = {"s": segT_both[:, 0:P], "t": segT_both[:, P:2 * P]}

            dps = ps_dot.tile([2, L], F32, space="PSUM", tag="dps")
            dots_sb = sb_s.tile([2, L], BF16, tag="dotsb", name="dotsb")
            dotsT = ps_sm.tile([P, 2 * NI], F32, space="PSUM", tag="dotsT",
                               name="dotsT")
            dt_ap = dotsT[:].rearrange("p (i s) -> p s i", s=2)
            dots_tok = {}
            for side in ("s", "t"):
                dots_tok[side] = sb_s.tile([P, NI], BF16, tag=f"dtk{side}",
                                           name=f"dtk{side}")

            # Segment-matmul accumulation uses ONE group for the whole shared
            # PSUM bank: start=True clears the entire bank's has_written bits,
            # so only the very first matmul may set it (first write per
            # element overwrites regardless).
            #
            # For all but the last batch: dots dense first, seg matmuls at the
            # end of the PE stream so their DVE dependencies cannot stall the
            # in-order PE FIFO.  For the LAST batch nothing runs behind it on
            # the PE, so stream per token group to minimize tail latency.
            last = (b == BLOC - 1)

            def emit_transp_seg(gl):
                for i in gl:
                    nc.tensor.matmul(out=dotsT[:, 2 * i:2 * i + 2],
                                     lhsT=dots_sb[:, i * P:(i + 1) * P],
                                     rhs=ident_bf[0:2, 0:2], start=True,
                                     stop=True)
                isl = slice(gl[0], gl[-1] + 1)
                nGI = len(gl)
                for sidx, side in enumerate(("s", "t")):
                    nc.vector.tensor_copy(out=dots_tok[side][:, isl],
                                          in_=dt_ap[:, sidx, isl])
                    nc.vector.tensor_tensor(
                        out=mdoq[side][:, isl, 0:Q],
                        in0=mdoq[side][:, isl, Q:2 * Q],
                        in1=dots_tok[side][:, isl].to_broadcast([P, nGI, Q]),
                        op=mybir.AluOpType.mult)
                for side in ("s", "t"):
                    for i in gl:
                        nc.tensor.matmul(out=segT[side],
                                         lhsT=mdoq[side][:, i, :],
                                         rhs=or_all[side][:, i, :],
                                         start=(side == "s" and i == 0),
                                         stop=(side == "t" and i == NI - 1))

            for g in range(NG):
                gs = slice(g * GL, (g + 1) * GL)
                for c in range(HC):
                    nc.tensor.matmul(out=dps[:, gs],
                                     lhsT=w2[:, c, :],
                                     rhs=htg[b][g][:, c, :],
                                     start=(c == 0), stop=(c == HC - 1))
                nc.scalar.copy(out=dots_sb[:, gs], in_=dps[:, gs])
                if last:
                    emit_transp_seg(list(range(g * GI, (g + 1) * GI)))
            if not last:
                emit_transp_seg(list(range(NI)))

            # ---- epilogue: ONE segT copy for both sides, two PE transposes
            # into adjacent seg4 regions, then BATCHED count/recip/divide
            # over both sides at once (halves the serial DVE hop count) ----
            segT_sb = sb_s.tile([2 * Q, 2 * P], F32, tag="segTsb",
                                name="segTsb")
            nc.scalar.copy(out=segT_sb[:], in_=segT_both[:])
            # seg4 per-batch layout: [t-sums Q | t-cnts Q | s-sums Q | s-cnts Q]
            ob = b * 4 * Q
            nc.tensor.transpose(out=seg4[:, ob:ob + 2 * Q],
                                in_=segT_sb[:, P:2 * P],
                                identity=ident[0:2 * Q, 0:2 * Q])
            nc.tensor.transpose(out=seg4[:, ob + 2 * Q:ob + 4 * Q],
                                in_=segT_sb[:, 0:P],
                                identity=ident[0:2 * Q, 0:2 * Q])
            # strided view [p, side', sums/cnts, q] over this batch's regions
            v4 = seg4[:].rearrange("p (n h q) -> p n h q", h=2, q=Q)
            cntb = sb_s.tile([P, 2, Q], F32, tag="cntb")
            nc.vector.tensor_scalar(out=cntb[:],
                                    in0=v4[:, 2 * b:2 * b + 2, 1, :],
                                    scalar1=1.0, scalar2=None,
                                    op0=mybir.AluOpType.max)
            recb = sb_s.tile([P, 2, Q], F32, tag="recb")
            nc.vector.reciprocal(out=recb[:], in_=cntb[:])
            projb = sb_s.tile([P, 2, Q], F32, tag="projb")
            nc.vector.tensor_tensor(out=projb[:],
                                    in0=v4[:, 2 * b:2 * b + 2, 0, :],
                                    in1=recb[:], op=mybir.AluOpType.mult)

            # fold bias into source projection (kept fp32: tensor_scalar
            # scalar operands must be fp32)
            proj_sb = sb_s.tile([P, Q], F32, tag="projsb")
            nc.vector.tensor_scalar(out=proj_sb[:], in0=projb[:, 1, :],
                                    scalar1=b_bc[:, 0:1], scalar2=None,
                                    op0=mybir.AluOpType.add)

            # ---- broadcast tgt projection to a [P, W] row:
            # tp[p, q*128+r] = proj_t[r, q].  8 tensor_scalar ops run in 4x
            # mode and take f32 scalars directly (no bf16 cast hop) ----
            msel = sb_s.tile([P, W], BF16, tag="msel")
            for qb in range(Q):
                nc.vector.tensor_scalar(
                    out=msel[:, qb * P:(qb + 1) * P], in0=ident_bf[:],
                    scalar1=projb[:, 0, qb:qb + 1], scalar2=None,
                    op0=mybir.AluOpType.mult)
            bc_sb = sb_s.tile([P, W], BF16, tag="bcsb")
            for half in range(2):
                bc_ps = ps_sm.tile([P, W // 2], F32, space="PSUM", tag="bc")
                nc.tensor.matmul(out=bc_ps[:], lhsT=ones_bf[:],
                                 rhs=msel[:, half * (W // 2):(half + 1) * (W // 2)],
                                 start=True, stop=True)
                nc.scalar.copy(out=bc_sb[:, half * (W // 2):(half + 1) * (W // 2)],
                               in_=bc_ps[:])

            # ---- output tiles: out[j*128+p, t] = proj_s[p, j] + tp[t] ----
            out_ap = out_d[b][:].rearrange("(j p) t -> p j t", p=P)
            if last:
                # latency-critical: store each tile as soon as its add is
                # done, spread across three DMA rings (incl. gpsimd SWDGE)
                for j in range(Q):
                    ot = sb_o.tile([P, W], BF16, tag="ot")
                    nc.vector.tensor_scalar(
                        out=ot[:], in0=bc_sb[:], scalar1=proj_sb[:, j:j + 1],
                        scalar2=None, op0=mybir.AluOpType.add)
                    eng = (nc.scalar, nc.sync, nc.gpsimd)[j % 3]
                    eng.dma_start(out=out_ap[:, j, :], in_=ot[:])
            else:
                # off the critical path: two coalesced 1 MiB stores (fewer
                # completion semaphores to sweep at kernel end)
                for hq in range(2):
                    ot4 = sb_o.tile([P, Q // 2, W], BF16, tag="ot4", bufs=1)
                    for jj in range(Q // 2):
                        j = hq * (Q // 2) + jj
                        nc.vector.tensor_scalar(
                            out=ot4[:, jj, :], in0=bc_sb[:],
                            scalar1=proj_sb[:, j:j + 1],
                            scalar2=None, op0=mybir.AluOpType.add)
                    nc.scalar.dma_start(
                        out=out_ap[:, hq * (Q // 2):(hq + 1) * (Q // 2), :],
                        in_=ot4[:])

    nc.compile()
    return nc, names


def _get_module():
    if "mod" not in _CACHE:
        _CACHE["mod"] = _build_module()
    return _CACHE["mod"]


def _run(hidden, classifier_w, classifier_b, source_word_ids, target_word_ids,
         **spmd_kwargs):
    nc, names = _get_module()
    bf16 = ml_dtypes.bfloat16
    hidden = np.asarray(hidden, dtype=np.float32)
    # swizzle to [p, g, c, tl] = hidden[b, g*GL+tl, c*128+p], bf16
    hidT = np.ascontiguousarray(
        hidden.astype(bf16).reshape(B, NG, GL, HC, P).transpose(0, 4, 1, 3, 2))
    wf = np.asarray(classifier_w, dtype=np.float32).reshape(2, H)  # [s|t], h
    # packed [P, 17]: cols 2c+s = w[s, c*128+p], col 16 = bias
    wb = np.empty((P, 2 * HC + 1), dtype=np.float32)
    wb[:, 0:2 * HC] = wf.reshape(2, HC, P).transpose(2, 1, 0).reshape(P, 2 * HC)
    wb[:, 2 * HC] = np.float32(np.asarray(classifier_b).reshape(-1)[0])
    src = np.ascontiguousarray(source_word_ids, dtype=np.int32)
    tgt = np.ascontiguousarray(target_word_ids, dtype=np.int32)

    in_maps = []
    for c in range(NCORES):
        ids_cat = np.empty((2 * BLOC, L), dtype=np.int32)
        for b in range(BLOC):
            gb = c * BLOC + b
            ids_cat[2 * b] = src[gb]
            ids_cat[2 * b + 1] = tgt[gb]
        # [k, i*128+p] -> [p, k, i]
        ids_sw = np.ascontiguousarray(
            ids_cat.reshape(2 * BLOC, NI, P).transpose(2, 0, 1))
        m = {names["wb"]: wb, names["ids"]: ids_sw}
        for b in range(BLOC):
            gb = c * BLOC + b
            m[names["hid"][b]] = hidT[gb]
        in_maps.append(m)

    res = run_bass_kernel_spmd(nc, in_maps, core_ids=list(range(NCORES)),
                               **spmd_kwargs)
    out = np.empty((B, W, W, 1), dtype=np.float32)
    for c in range(NCORES):
        for b in range(BLOC):
            out[c * BLOC + b, :, :, 0] = res.results[c][names["out"][b]].astype(
                np.float32)
    return out, res


def kernel(hidden, classifier_w, classifier_b, source_word_ids,
           target_word_ids, num_words):
    out, _ = _run(hidden, classifier_w, classifier_b, source_word_ids,
                  target_word_ids)
    return out
